# revision 1
# baseline (speedup 1.0000x reference)
"""Trainium2 Bass kernel for nn_LogicConvSparseMatrix.

Math: the reference's 15-term weighted logic-op sum collapses to

    out[b,k] = C_ab[k]*A*B + C_a[k]*A + C_b[k]*B + C_1[k]

where A = x[b, ca_k, ha_k+oh, wa_k+ow], B = x[b, cb_k, hb_k+oh, wb_k+ow]
are shifted 126x126 windows.  With alpha = C_b/C_ab, gamma = C_1 -
C_a*C_b/C_ab this factors into

    out = (A + alpha) * (C_ab*B + C_a) + gamma

Per kernel k (three element passes; two ops cannot carry 4 coefficients):
  1. ScalarE affine:  B2 = C_ab*B + C_a
  2. VectorE scalar_tensor_tensor:  T = (A + alpha) * B2
  3. "+gamma", load-balanced per group of 8 k's across:
       - ScalarE Copy(T*1 + gamma) in place,
       - VectorE tensor_scalar add (AP shaped [4,63] to force 1x mode so
         it never grabs the DVE/GpSimd shared SBUF port), or
       - GpSimd tensor_tensor T + gcol (broadcast gamma table; GpSimd's
         tensor_scalar kernel is pathologically slow, tensor_tensor is ok).

Index pairs are known at build time, so gathers are compile-time SBUF
views of X[p=h, (c,b,w)].  Compute-engine SBUF operands may only start
at partition 0/32/64/96; the relative h-shift between the two windows is
materialized as shifted column copies via SBUF->SBUF DMA (DMA may
address any partition), consolidated into gap-bridged contiguous
channel-range runs (one DMA each).  All compute APs start at partition
0; store DMAs select rows [base : base+126].

k's are processed sorted by base so stores batch into ~1MB run DMAs
issued from the (otherwise idle) GpSimd queue via SWDGE, whose issue
cost is ~0.7us and whose transfers run async; HWDGE queue transfers
block their issuing engine, so loads/shift-copies are split between the
SP queue (batch 0 + copies) and Activation queue (batch 1, issued while
ScalarE is still idle).  The device output layout is [K, BPC, OH, OW]
with k's in base-sorted order; the host inverse-permutes/transposes.
Sharding: data-parallel over batch, 2 batch items per core, 8 cores.
"""

import numpy as np

B, C, H, W = 16, 64, 128, 128
K = 128
RH = RW = 3
OH, OW = H - RH + 1, W - RW + 1
NCORES = 8
BPC = B // NCORES

GRP = 8  # kernels per store group
GSPLIT = ("gp", "gp", "dve", "act")  # gamma-engine per group, round-robin


def _coeffs(weights):
    """Per-kernel coefficients of out = Cab*a*b + Ca*a + Cb*b + C1."""
    w = [weights[:, i].astype(np.float64) for i in range(16)]
    cab = w[1] - w[2] - w[4] - 2 * w[6] - w[7] + w[8] + 2 * w[9] + w[11] + w[13] - w[14]
    ca = w[2] + w[3] + w[6] + w[7] - w[8] - w[9] - w[12] - w[13]
    cb = w[4] + w[5] + w[6] + w[7] - w[8] - w[9] - w[10] - w[11]
    c1 = w[8] + w[9] + w[10] + w[11] + w[12] + w[13] + w[14] + w[15]
    return cab, ca, cb, c1


def _plan(pairs_a, pairs_b, weights):
    """Host-side schedule.  Returns (plans, runs, order, gcol) where
    plans[k] = (k, base, a_src, b_src, path, scal, gamma) with
    a_src/b_src = (from_shifted, column_index, w_off), runs = list of
    (shift, c0, c1, dest_col0) shifted-copy DMAs plus total column count,
    order = base-sorted k order, gcol = broadcast gamma table."""
    cab, ca, cb, c1 = _coeffs(weights)
    keys = {}  # (shift, chan) -> use count; shift != 0
    raw = []
    for k in range(K):
        ha, wa, cca = int(pairs_a[k][0]), int(pairs_a[k][1]), int(pairs_a[k][2])
        hb, wb, ccb = int(pairs_b[k][0]), int(pairs_b[k][1]), int(pairs_b[k][2])
        if ha == hb:
            base = ha
            a_key, b_key = (0, cca), (0, ccb)
        else:
            # shifting either side keeps that copy's invalid rows inside the
            # junk-lane range (min_h + |delta| <= 2); reuse existing columns.
            if ha < hb:  # a is the smaller-h side
                neg = ((ha - hb, cca), True, hb)  # (col key, shifts_a, base)
                pos = ((hb - ha, ccb), False, ha)
            else:
                neg = ((hb - ha, ccb), False, ha)
                pos = ((ha - hb, cca), True, hb)
            key, shift_a, base = pos if (pos[0] in keys and neg[0] not in keys) else neg
            keys[key] = keys.get(key, 0) + 1
            if shift_a:
                a_key, b_key = key, (0, ccb)
            else:
                a_key, b_key = (0, cca), key

        kab, kka, kkb, kk1 = float(cab[k]), float(ca[k]), float(cb[k]), float(c1[k])
        if abs(kab) <= 1e-7:
            path, scal, gamma = "linear", (kka, kkb, kk1), 0.0
        elif abs(kkb) <= 50.0 * abs(kab) and abs(kka * kkb) <= 50.0 * abs(kab):
            path = "fact"
            scal = (kab, kka, kkb / kab)
            gamma = kk1 - kka * kkb / kab
        else:
            path, scal, gamma = "exact", (kab, kka, kkb, kk1), 0.0
        raw.append((k, base, a_key, wa, b_key, wb, path, scal, gamma))

    # consolidate shifted columns into gap-bridged contiguous c-runs
    def build_runs(gaptol):
        runs, cmap, total = [], {}, 0
        for s in sorted({sc[0] for sc in keys}):
            cs = sorted(c for (s2, c) in keys if s2 == s)
            i = 0
            while i < len(cs):
                j = i
                while j + 1 < len(cs) and cs[j + 1] - cs[j] <= gaptol:
                    j += 1
                c0, cl = cs[i], cs[j]
                for c in range(c0, cl + 1):
                    cmap[(s, c)] = total + (c - c0)
                runs.append((s, c0, cl, total))
                total += cl - c0 + 1
                i = j + 1
        return runs, cmap, total

    for gaptol in (8, 4, 1, 0):
        runlist, cmap, ncols = build_runs(gaptol)
        if ncols <= 75:
            break

    plans = []
    for (k, base, a_key, wa, b_key, wb, path, scal, gamma) in raw:
        a_src = (False, a_key[1], wa) if a_key[0] == 0 else (True, cmap[a_key], wa)
        b_src = (False, b_key[1], wb) if b_key[0] == 0 else (True, cmap[b_key], wb)
        plans.append((k, base, a_src, b_src, path, scal, gamma))

    order = sorted(
        range(K), key=lambda k: (plans[k][1], plans[k][2][0] or plans[k][3][0], k)
    )  # by base, no-shift kernels first within each base run
    gcol = np.zeros((H, K), np.float32)
    for pos, k in enumerate(order):
        gcol[:, pos] = plans[k][6]
    return plans, (runlist, ncols), order, gcol


def _build(pairs_a, pairs_b, weights):
    import concourse.bacc as bacc
    import concourse.mybir as mybir
    from concourse.tile import TileContext

    f32 = mybir.dt.float32
    Copy = mybir.ActivationFunctionType.Copy
    add, mult = mybir.AluOpType.add, mybir.AluOpType.mult

    plans, (runlist, ncols), order, gcol_np = _plan(pairs_a, pairs_b, weights)
    ncols = max(1, ncols)
    ngrp = (K + GRP - 1) // GRP

    if ncols > 80:
        raise RuntimeError(f"shifted-column budget exceeded: {ncols}")

    nc = bacc.Bacc()
    x = nc.dram_tensor("x", [C, H, BPC, W], f32, kind="ExternalInput")
    gcd = nc.dram_tensor("gcol", [H, K], f32, kind="ExternalInput")
    out = nc.dram_tensor("out", [K, BPC, OH, OW], f32, kind="ExternalOutput")

    with TileContext(nc) as tc:
        with (
            tc.tile_pool(name="xp", bufs=1) as xp,
            tc.tile_pool(name="bp", bufs=6) as bp,
            tc.tile_pool(name="tp", bufs=3) as tp,
            tc.tile_pool(name="op", bufs=2) as op,
        ):
            # x arrives host-transposed as [C, H, BPC, W] so both the main
            # staging load and the shifted-run loads are straight 3-dim
            # DRAM->SBUF DMAs (shifted SBUF->SBUF copies measured ~40 GB/s).
            xr = x.rearrange("c h b w -> h c (b w)")
            X = xp.tile([H, C * BPC * W], f32)
            Xv = X.rearrange("p (c b w) -> p c b w", c=C, b=BPC)
            Xf = X.rearrange("p (c q) -> p c q", c=C)
            half = C // 2
            nc.sync.dma_start(out=Xf[:, 0:half], in_=xr[:, 0:half])
            nc.sync.dma_start(out=Xf[:, half:C], in_=xr[:, half:C])

            S = xp.tile([H, ncols * BPC * W], f32)
            Sv = S.rearrange("p (j b w) -> p j b w", j=ncols, b=BPC)
            Sf = S.rearrange("p (j q) -> p j q", j=ncols)
            # finite filler for shifted-run head/tail rows (junk lanes only)
            for d0 in range(0, ncols, C):
                n = min(C, ncols - d0)
                nc.sync.dma_start(out=Sf[0:2, d0 : d0 + n], in_=xr[0:2, 0:n])
                nc.sync.dma_start(out=Sf[H - 2 : H, d0 : d0 + n], in_=xr[0:2, 0:n])
            for ri, (s, c0, cl, d0) in enumerate(runlist):
                # S[p, d0+i] = x[c0+i, p+s], loaded from DRAM.  All loads stay
                # on the SP queue: a compute engine's stream blocks on its own
                # queue's transfers, so Activation must carry no DMAs.
                eng = nc.sync
                n = cl - c0 + 1
                if s < 0:
                    eng.dma_start(
                        out=Sf[-s:H, d0 : d0 + n], in_=xr[0 : H + s, c0 : c0 + n]
                    )
                else:
                    eng.dma_start(
                        out=Sf[0 : H - s, d0 : d0 + n], in_=xr[s:H, c0 : c0 + n]
                    )

            Gc = xp.tile([H, K], f32)
            nc.sync.dma_start(out=Gc, in_=gcd[:, :])

            out_kb = out.rearrange("k b oh ow -> (k b) oh ow")
            fd = BPC * OW

            def emit_gamma_and_store(g, ks, geng, T, O):
                # deferred one group so cross-engine waits are pre-satisfied
                for j, k in enumerate(ks):
                    _, base, _, _, path, scal, gamma = plans[k]
                    cnt = base + OH
                    slot = T[0:cnt, j * fd : (j + 1) * fd]
                    if gamma != 0.0 or geng == "gp":
                        pos = g * GRP + j
                        if geng == "act":
                            nc.scalar.activation(
                                slot, slot, Copy, bias=gamma, scale=1.0
                            )
                        elif geng == "dve":
                            # odd innermost dim forces 1x mode: no shared-port
                            # contention with GpSimd
                            so = slot.rearrange("p (a q) -> p a q", a=4)
                            nc.vector.tensor_scalar(so, so, gamma, None, add)
                        else:
                            gb = Gc[0:cnt, pos : pos + 1].broadcast_to([cnt, fd])
                            osl = O[0:cnt, j * fd : (j + 1) * fd]
                            nc.gpsimd.tensor_tensor(osl, slot, gb, add)
                # batched stores per same-base run: SWDGE on the GpSimd queue
                # (issue ~0.7us, transfer async; HWDGE would block its engine).
                src_t = O if geng == "gp" else T
                i = 0
                while i < len(ks):
                    base = plans[ks[i]][1]
                    i2 = i
                    while i2 < len(ks) and plans[ks[i2]][1] == base:
                        i2 += 1
                    src = src_t[base : base + OH, i * fd : i2 * fd].rearrange(
                        "p (kb w) -> p kb w", w=OW
                    )
                    dst = out_kb[(g * GRP + i) * BPC : (g * GRP + i2) * BPC]
                    nc.gpsimd.dma_start(
                        out=dst.rearrange("kb oh ow -> oh kb ow"), in_=src
                    )
                    i = i2

            pending = None
            for g in range(ngrp):
                ks = order[g * GRP : (g + 1) * GRP]
                geng = GSPLIT[g % len(GSPLIT)]
                T = tp.tile([H, GRP * fd], f32, tag="t", name=f"t_{g}")
                O = None
                if geng == "gp":
                    O = op.tile([H, GRP * fd], f32, tag="o", name=f"o_{g}")

                for j, k in enumerate(ks):
                    _, base, a_src, b_src, path, scal, gamma = plans[k]
                    cnt = base + OH

                    def view(src):
                        shifted, idx, woff = src
                        t = Sv if shifted else Xv
                        return t[0:cnt, idx, :, woff : woff + OW]

                    Av, Bv = view(a_src), view(b_src)
                    slot = T[0:cnt, j * fd : (j + 1) * fd]
                    slotv = slot.rearrange("p (b w) -> p b w", b=BPC)
                    b2 = bp.tile([H, fd], f32, tag="b2", name=f"b2_{k}")
                    b2v = b2.rearrange("p (b w) -> p b w", b=BPC)[0:cnt]

                    if path == "fact":
                        kab, kka, alpha = scal
                        nc.scalar.activation(b2v, Bv, Copy, bias=kka, scale=kab)
                        nc.vector.scalar_tensor_tensor(slotv, Av, alpha, b2v, add, mult)
                    else:  # linear/exact: slot = Ca*A + (Cb*B + C1)
                        if path == "linear":
                            kka, kkb, kk1 = scal
                        else:
                            kab, kka, kkb, kk1 = scal
                        nc.scalar.activation(b2v, Bv, Copy, bias=kk1, scale=kkb)
                        nc.vector.scalar_tensor_tensor(slotv, Av, kka, b2v, mult, add)
                        if path == "exact":  # += (Cab*B)*A
                            p2 = bp.tile([H, fd], f32, tag="b2", name=f"p2_{k}")
                            p2v = p2.rearrange("p (b w) -> p b w", b=BPC)[0:cnt]
                            nc.vector.scalar_tensor_tensor(p2v, Bv, kab, Av, mult, mult)
                            nc.vector.tensor_tensor(slot, slot, p2[0:cnt], add)

                if pending is not None:
                    emit_gamma_and_store(*pending)
                pending = (g, ks, geng, T, O)
            if pending is not None:
                emit_gamma_and_store(*pending)
    nc.compile()
    return nc


def _consts(pairs_a, pairs_b, weights):
    plans, runs, order, gcol = _plan(pairs_a, pairs_b, weights)
    return {"gcol": gcol}, order


def kernel(x, pairs_a, pairs_b, weights):
    from concourse.bass_utils import run_bass_kernel_spmd

    x = np.ascontiguousarray(np.asarray(x), dtype=np.float32)
    pa = np.asarray(pairs_a).astype(np.int64)
    pb = np.asarray(pairs_b).astype(np.int64)
    w = np.asarray(weights).astype(np.float32)

    nc = _build(pa, pb, w)
    extra, order = _consts(pa, pb, w)
    in_maps = [
        {
            "x": np.ascontiguousarray(
                x[i * BPC : (i + 1) * BPC].transpose(1, 2, 0, 3)
            ),
            **extra,
        }
        for i in range(NCORES)
    ]
    res = run_bass_kernel_spmd(nc, in_maps, core_ids=list(range(NCORES)))
    # device layout [K(sorted), BPC, OH, OW] per core -> [B, K, OH, OW]
    full = np.concatenate([r["out"] for r in res.results], axis=1)  # [K, B, ...]
    pos = np.empty(K, np.int64)
    pos[np.asarray(order)] = np.arange(K)
    return np.ascontiguousarray(full[pos].transpose(1, 0, 2, 3))



# revision 7
# speedup vs baseline: 1.1215x; 1.1215x over previous
"""Trainium2 Bass kernel for nn_LogicConvSparseMatrix.

Math: the reference's 15-term weighted logic-op sum collapses to

    out[b,k] = C_ab[k]*A*B + C_a[k]*A + C_b[k]*B + C_1[k]

where A = x[b, ca_k, ha_k+oh, wa_k+ow], B = x[b, cb_k, hb_k+oh, wb_k+ow]
are shifted 126x126 windows.  With alpha = C_b/C_ab, gamma = C_1 -
C_a*C_b/C_ab this factors into

    out = (A + alpha) * (C_ab*B + C_a) + gamma

Per kernel k (three element passes; two ops cannot carry 4 coefficients):
  1. ScalarE affine:  B2 = C_ab*B + C_a
  2. VectorE scalar_tensor_tensor:  T = (A + alpha) * B2
  3. "+gamma", load-balanced per group of 8 k's across:
       - ScalarE Copy(T*1 + gamma) in place,
       - VectorE tensor_scalar add (AP shaped [4,63] to force 1x mode so
         it never grabs the DVE/GpSimd shared SBUF port), or
       - GpSimd tensor_tensor T + gcol (broadcast gamma table; GpSimd's
         tensor_scalar kernel is pathologically slow, tensor_tensor is ok).

Index pairs are known at build time, so gathers are compile-time SBUF
views of X[p=h, (c,b,w)].  Compute-engine SBUF operands may only start
at partition 0/32/64/96; the relative h-shift between the two windows is
materialized as shifted column copies via SBUF->SBUF DMA (DMA may
address any partition), consolidated into gap-bridged contiguous
channel-range runs (one DMA each).  All compute APs start at partition
0; store DMAs select rows [base : base+126].

k's are processed sorted by base so stores batch into ~1MB run DMAs
issued from the (otherwise idle) GpSimd queue via SWDGE, whose issue
cost is ~0.7us and whose transfers run async; HWDGE queue transfers
block their issuing engine, so loads/shift-copies are split between the
SP queue (batch 0 + copies) and Activation queue (batch 1, issued while
ScalarE is still idle).  The device output layout is [K, BPC, OH, OW]
with k's in base-sorted order; the host inverse-permutes/transposes.
Sharding: data-parallel over batch, 2 batch items per core, 8 cores.
"""

import numpy as np

B, C, H, W = 16, 64, 128, 128
K = 128
RH = RW = 3
OH, OW = H - RH + 1, W - RW + 1
NCORES = 8
BPC = B // NCORES

GRP = 8  # kernels per store group
GSPLIT = ("gp", "dve", "act", "gp", "dve", "act", "gp", "dve")  # gamma engine


def _coeffs(weights):
    """Per-kernel coefficients of out = Cab*a*b + Ca*a + Cb*b + C1."""
    w = [weights[:, i].astype(np.float64) for i in range(16)]
    cab = w[1] - w[2] - w[4] - 2 * w[6] - w[7] + w[8] + 2 * w[9] + w[11] + w[13] - w[14]
    ca = w[2] + w[3] + w[6] + w[7] - w[8] - w[9] - w[12] - w[13]
    cb = w[4] + w[5] + w[6] + w[7] - w[8] - w[9] - w[10] - w[11]
    c1 = w[8] + w[9] + w[10] + w[11] + w[12] + w[13] + w[14] + w[15]
    return cab, ca, cb, c1


def _plan(pairs_a, pairs_b, weights):
    """Host-side schedule.  Returns (plans, runs, order, gcol) where
    plans[k] = (k, base, a_src, b_src, path, scal, gamma) with
    a_src/b_src = (from_shifted, column_index, w_off), runs = list of
    (shift, c0, c1, dest_col0) shifted-copy DMAs plus total column count,
    order = base-sorted k order, gcol = broadcast gamma table."""
    cab, ca, cb, c1 = _coeffs(weights)
    keys = {}  # (shift, chan) -> use count; shift != 0
    raw = []
    for k in range(K):
        ha, wa, cca = int(pairs_a[k][0]), int(pairs_a[k][1]), int(pairs_a[k][2])
        hb, wb, ccb = int(pairs_b[k][0]), int(pairs_b[k][1]), int(pairs_b[k][2])
        if ha == hb:
            base = ha
            a_key, b_key = (0, cca), (0, ccb)
        else:
            # shifting either side keeps that copy's invalid rows inside the
            # junk-lane range (min_h + |delta| <= 2); reuse existing columns.
            if ha < hb:  # a is the smaller-h side
                neg = ((ha - hb, cca), True, hb)  # (col key, shifts_a, base)
                pos = ((hb - ha, ccb), False, ha)
            else:
                neg = ((hb - ha, ccb), False, ha)
                pos = ((ha - hb, cca), True, hb)
            key, shift_a, base = pos if (pos[0] in keys and neg[0] not in keys) else neg
            keys[key] = keys.get(key, 0) + 1
            if shift_a:
                a_key, b_key = key, (0, ccb)
            else:
                a_key, b_key = (0, cca), key

        kab, kka, kkb, kk1 = float(cab[k]), float(ca[k]), float(cb[k]), float(c1[k])
        if abs(kab) <= 1e-7:
            path, scal, gamma = "linear", (kka, kkb, kk1), 0.0
        elif abs(kkb) <= 50.0 * abs(kab) and abs(kka * kkb) <= 50.0 * abs(kab):
            path = "fact"
            scal = (kab, kka, kkb / kab)
            gamma = kk1 - kka * kkb / kab
        else:
            path, scal, gamma = "exact", (kab, kka, kkb, kk1), 0.0
        raw.append((k, base, a_key, wa, b_key, wb, path, scal, gamma))

    # consolidate shifted columns into gap-bridged contiguous c-runs
    def build_runs(gaptol):
        runs, cmap, total = [], {}, 0
        for s in sorted({sc[0] for sc in keys}):
            cs = sorted(c for (s2, c) in keys if s2 == s)
            i = 0
            while i < len(cs):
                j = i
                while j + 1 < len(cs) and cs[j + 1] - cs[j] <= gaptol:
                    j += 1
                c0, cl = cs[i], cs[j]
                for c in range(c0, cl + 1):
                    cmap[(s, c)] = total + (c - c0)
                runs.append((s, c0, cl, total))
                total += cl - c0 + 1
                i = j + 1
        return runs, cmap, total

    for gaptol in (8, 4, 1, 0):
        runlist, cmap, ncols = build_runs(gaptol)
        if ncols <= 75:
            break

    plans = []
    for (k, base, a_key, wa, b_key, wb, path, scal, gamma) in raw:
        a_src = (False, a_key[1], wa) if a_key[0] == 0 else (True, cmap[a_key], wa)
        b_src = (False, b_key[1], wb) if b_key[0] == 0 else (True, cmap[b_key], wb)
        plans.append((k, base, a_src, b_src, path, scal, gamma))

    order = sorted(
        range(K), key=lambda k: (plans[k][1], plans[k][2][0] or plans[k][3][0], k)
    )  # by base, no-shift kernels first within each base run
    gcol = np.zeros((H, K), np.float32)
    for pos, k in enumerate(order):
        gcol[:, pos] = plans[k][6]
    return plans, (runlist, ncols), order, gcol


def _build(pairs_a, pairs_b, weights):
    import concourse.bacc as bacc
    import concourse.mybir as mybir
    from concourse.tile import TileContext

    f32 = mybir.dt.float32
    Copy = mybir.ActivationFunctionType.Copy
    add, mult = mybir.AluOpType.add, mybir.AluOpType.mult

    plans, (runlist, ncols), order, gcol_np = _plan(pairs_a, pairs_b, weights)
    ncols = max(1, ncols)
    ngrp = (K + GRP - 1) // GRP

    if ncols > 80:
        raise RuntimeError(f"shifted-column budget exceeded: {ncols}")

    nc = bacc.Bacc()
    # h-major DRAM layout: each SBUF partition row of the main load is one
    # contiguous 64KB DRAM span (the former [C,H,...] layout produced 1KB
    # descriptors and measured ~74 GB/s; h-major reaches near line rate).
    x = nc.dram_tensor("x", [H, C, BPC, W], f32, kind="ExternalInput")
    gcd = nc.dram_tensor("gcol", [H, K], f32, kind="ExternalInput")
    # oh-major output: a store from T[base:base+OH] writes, per partition,
    # one contiguous (k-run x BPC x OW) span (~8KB for a full group) instead
    # of 504B per (k,b).  Host transposes back.
    out = nc.dram_tensor("out", [OH, K, BPC, OW], f32, kind="ExternalOutput")

    with TileContext(nc) as tc:
        with (
            tc.tile_pool(name="xp", bufs=1) as xp,
            tc.tile_pool(name="bp", bufs=6) as bp,
            tc.tile_pool(name="tp", bufs=3) as tp,
            tc.tile_pool(name="op", bufs=2) as op,
        ):
            # x arrives host-transposed as [H, C, BPC, W]: per-partition
            # contiguous loads.  Chunked over c so compute on early channels
            # can start while later chunks stream; two chunks ride the
            # otherwise-idle Activation HWDGE ring in parallel with SP.
            xr = x.rearrange("h c b w -> h c (b w)")
            X = xp.tile([H, C * BPC * W], f32)
            Xv = X.rearrange("p (c b w) -> p c b w", c=C, b=BPC)
            Xf = X.rearrange("p (c q) -> p c q", c=C)
            qtr = C // 4
            for ci in range(4):
                eng = nc.sync if ci % 2 == 0 else nc.scalar
                eng.dma_start(
                    out=Xf[:, ci * qtr : (ci + 1) * qtr],
                    in_=xr[:, ci * qtr : (ci + 1) * qtr],
                )

            S = xp.tile([H, ncols * BPC * W], f32)
            Sv = S.rearrange("p (j b w) -> p j b w", j=ncols, b=BPC)
            Sf = S.rearrange("p (j q) -> p j q", j=ncols)
            # finite filler for shifted-run head/tail rows (junk lanes only)
            for d0 in range(0, ncols, C):
                n = min(C, ncols - d0)
                nc.sync.dma_start(out=Sf[0:2, d0 : d0 + n], in_=xr[0:2, 0:n])
                nc.sync.dma_start(out=Sf[H - 2 : H, d0 : d0 + n], in_=xr[0:2, 0:n])
            for ri, (s, c0, cl, d0) in enumerate(runlist):
                # S[p, d0+i] = x[c0+i, p+s], loaded from DRAM.  All loads stay
                # on the SP queue: a compute engine's stream blocks on its own
                # queue's transfers, so Activation must carry no DMAs.
                eng = nc.sync
                n = cl - c0 + 1
                if s < 0:
                    eng.dma_start(
                        out=Sf[-s:H, d0 : d0 + n], in_=xr[0 : H + s, c0 : c0 + n]
                    )
                else:
                    eng.dma_start(
                        out=Sf[0 : H - s, d0 : d0 + n], in_=xr[s:H, c0 : c0 + n]
                    )

            Gc = xp.tile([H, K], f32)
            nc.sync.dma_start(out=Gc, in_=gcd[:, :])

            out_kb = out.rearrange("oh k b ow -> oh (k b) ow")
            fd = BPC * OW

            def emit_gamma_and_store(g, ks, geng, T, O):
                # deferred one group so cross-engine waits are pre-satisfied
                for j, k in enumerate(ks):
                    _, base, _, _, path, scal, gamma = plans[k]
                    cnt = base + OH
                    slot = T[0:cnt, j * fd : (j + 1) * fd]
                    if gamma != 0.0 or geng == "gp":
                        pos = g * GRP + j
                        if geng == "act":
                            nc.scalar.activation(
                                slot, slot, Copy, bias=gamma, scale=1.0
                            )
                        elif geng == "dve":
                            # odd innermost dim forces 1x mode: no shared-port
                            # contention with GpSimd
                            so = slot.rearrange("p (a q) -> p a q", a=4)
                            nc.vector.tensor_scalar(so, so, gamma, None, add)
                        else:
                            gb = Gc[0:cnt, pos : pos + 1].broadcast_to([cnt, fd])
                            osl = O[0:cnt, j * fd : (j + 1) * fd]
                            nc.gpsimd.tensor_tensor(osl, slot, gb, add)
                # batched stores per same-base run: SWDGE on the GpSimd queue
                # (issue ~0.7us, transfer async; HWDGE would block its engine).
                src_t = O if geng == "gp" else T
                i = 0
                while i < len(ks):
                    base = plans[ks[i]][1]
                    i2 = i
                    while i2 < len(ks) and plans[ks[i2]][1] == base:
                        i2 += 1
                    src = src_t[base : base + OH, i * fd : i2 * fd].rearrange(
                        "p (kb w) -> p kb w", w=OW
                    )
                    dst = out_kb[:, (g * GRP + i) * BPC : (g * GRP + i2) * BPC]
                    nc.gpsimd.dma_start(out=dst, in_=src)
                    i = i2

            pending = None
            for g in range(ngrp):
                ks = order[g * GRP : (g + 1) * GRP]
                geng = GSPLIT[g % len(GSPLIT)]
                T = tp.tile([H, GRP * fd], f32, tag="t", name=f"t_{g}")
                O = None
                if geng == "gp":
                    O = op.tile([H, GRP * fd], f32, tag="o", name=f"o_{g}")

                for j, k in enumerate(ks):
                    _, base, a_src, b_src, path, scal, gamma = plans[k]
                    cnt = base + OH

                    def view(src):
                        shifted, idx, woff = src
                        t = Sv if shifted else Xv
                        return t[0:cnt, idx, :, woff : woff + OW]

                    Av, Bv = view(a_src), view(b_src)
                    slot = T[0:cnt, j * fd : (j + 1) * fd]
                    slotv = slot.rearrange("p (b w) -> p b w", b=BPC)
                    b2 = bp.tile([H, fd], f32, tag="b2", name=f"b2_{k}")
                    b2v = b2.rearrange("p (b w) -> p b w", b=BPC)[0:cnt]

                    if path == "fact":
                        kab, kka, alpha = scal
                        nc.scalar.activation(b2v, Bv, Copy, bias=kka, scale=kab)
                        nc.vector.scalar_tensor_tensor(slotv, Av, alpha, b2v, add, mult)
                    else:  # linear/exact: slot = Ca*A + (Cb*B + C1)
                        if path == "linear":
                            kka, kkb, kk1 = scal
                        else:
                            kab, kka, kkb, kk1 = scal
                        nc.scalar.activation(b2v, Bv, Copy, bias=kk1, scale=kkb)
                        nc.vector.scalar_tensor_tensor(slotv, Av, kka, b2v, mult, add)
                        if path == "exact":  # += (Cab*B)*A
                            p2 = bp.tile([H, fd], f32, tag="b2", name=f"p2_{k}")
                            p2v = p2.rearrange("p (b w) -> p b w", b=BPC)[0:cnt]
                            nc.vector.scalar_tensor_tensor(p2v, Bv, kab, Av, mult, mult)
                            nc.vector.tensor_tensor(slot, slot, p2[0:cnt], add)

                if pending is not None:
                    emit_gamma_and_store(*pending)
                pending = (g, ks, geng, T, O)
            if pending is not None:
                emit_gamma_and_store(*pending)
    nc.compile()
    return nc


def _consts(pairs_a, pairs_b, weights):
    plans, runs, order, gcol = _plan(pairs_a, pairs_b, weights)
    return {"gcol": gcol}, order


def kernel(x, pairs_a, pairs_b, weights):
    from concourse.bass_utils import run_bass_kernel_spmd

    x = np.ascontiguousarray(np.asarray(x), dtype=np.float32)
    pa = np.asarray(pairs_a).astype(np.int64)
    pb = np.asarray(pairs_b).astype(np.int64)
    w = np.asarray(weights).astype(np.float32)

    nc = _build(pa, pb, w)
    extra, order = _consts(pa, pb, w)
    in_maps = [
        {
            "x": np.ascontiguousarray(
                x[i * BPC : (i + 1) * BPC].transpose(2, 1, 0, 3)
            ),
            **extra,
        }
        for i in range(NCORES)
    ]
    res = run_bass_kernel_spmd(nc, in_maps, core_ids=list(range(NCORES)))
    # device layout [OH, K(sorted), BPC, OW] per core -> [B, K, OH, OW]
    full = np.concatenate([r["out"] for r in res.results], axis=2)  # [OH,K,B,OW]
    pos = np.empty(K, np.int64)
    pos[np.asarray(order)] = np.arange(K)
    return np.ascontiguousarray(full[:, pos].transpose(2, 1, 0, 3))



# revision 8
# speedup vs baseline: 1.6647x; 1.4844x over previous
"""Trainium2 Bass kernel for nn_LogicConvSparseMatrix.

Math: the reference's 15-term weighted logic-op sum collapses to

    out[b,k] = C_ab[k]*A*B + C_a[k]*A + C_b[k]*B + C_1[k]

where A = x[b, ca_k, ha_k+oh, wa_k+ow], B = x[b, cb_k, hb_k+oh, wb_k+ow]
are shifted 126x126 windows.  With alpha = C_b/C_ab, gamma = C_1 -
C_a*C_b/C_ab this factors into

    out = (A + alpha) * (C_ab*B + C_a) + gamma

Per kernel k (three element passes; two ops cannot carry 4 coefficients):
  1. ScalarE affine:  B2 = C_ab*B + C_a
  2. VectorE scalar_tensor_tensor:  T = (A + alpha) * B2
  3. "+gamma", load-balanced per group of 8 k's across:
       - ScalarE Copy(T*1 + gamma) in place,
       - VectorE tensor_scalar add (AP shaped [4,63] to force 1x mode so
         it never grabs the DVE/GpSimd shared SBUF port), or
       - GpSimd tensor_tensor T + gcol (broadcast gamma table; GpSimd's
         tensor_scalar kernel is pathologically slow, tensor_tensor is ok).

Index pairs are known at build time, so gathers are compile-time SBUF
views of X[p=h, (c,b,w)].  Compute-engine SBUF operands may only start
at partition 0/32/64/96; the relative h-shift between the two windows is
materialized as shifted column copies via SBUF->SBUF DMA (DMA may
address any partition), consolidated into gap-bridged contiguous
channel-range runs (one DMA each).  All compute APs start at partition
0; store DMAs select rows [base : base+126].

k's are processed sorted by base so stores batch into ~1MB run DMAs
issued from the (otherwise idle) GpSimd queue via SWDGE, whose issue
cost is ~0.7us and whose transfers run async; HWDGE queue transfers
block their issuing engine, so loads/shift-copies are split between the
SP queue (batch 0 + copies) and Activation queue (batch 1, issued while
ScalarE is still idle).  The device output layout is [K, BPC, OH, OW]
with k's in base-sorted order; the host inverse-permutes/transposes.
Sharding: data-parallel over batch, 2 batch items per core, 8 cores.
"""

import numpy as np

B, C, H, W = 16, 64, 128, 128
K = 128
RH = RW = 3
OH, OW = H - RH + 1, W - RW + 1
NCORES = 8
BPC = B // NCORES

GRP = 8  # kernels per store group
GSPLIT = ("gp", "dve", "act", "gp", "dve", "act", "gp", "dve")  # gamma engine


def _coeffs(weights):
    """Per-kernel coefficients of out = Cab*a*b + Ca*a + Cb*b + C1."""
    w = [weights[:, i].astype(np.float64) for i in range(16)]
    cab = w[1] - w[2] - w[4] - 2 * w[6] - w[7] + w[8] + 2 * w[9] + w[11] + w[13] - w[14]
    ca = w[2] + w[3] + w[6] + w[7] - w[8] - w[9] - w[12] - w[13]
    cb = w[4] + w[5] + w[6] + w[7] - w[8] - w[9] - w[10] - w[11]
    c1 = w[8] + w[9] + w[10] + w[11] + w[12] + w[13] + w[14] + w[15]
    return cab, ca, cb, c1


def _plan(pairs_a, pairs_b, weights):
    """Host-side schedule.  Returns (plans, runs, order, gcol) where
    plans[k] = (k, base, a_src, b_src, path, scal, gamma) with
    a_src/b_src = (from_shifted, column_index, w_off), runs = list of
    (shift, c0, c1, dest_col0) shifted-copy DMAs plus total column count,
    order = base-sorted k order, gcol = broadcast gamma table."""
    cab, ca, cb, c1 = _coeffs(weights)
    keys = {}  # (shift, chan) -> use count; shift != 0
    raw = []
    for k in range(K):
        ha, wa, cca = int(pairs_a[k][0]), int(pairs_a[k][1]), int(pairs_a[k][2])
        hb, wb, ccb = int(pairs_b[k][0]), int(pairs_b[k][1]), int(pairs_b[k][2])
        if ha == hb:
            base = ha
            a_key, b_key = (0, cca), (0, ccb)
        else:
            # shifting either side keeps that copy's invalid rows inside the
            # junk-lane range (min_h + |delta| <= 2); reuse existing columns.
            if ha < hb:  # a is the smaller-h side
                neg = ((ha - hb, cca), True, hb)  # (col key, shifts_a, base)
                pos = ((hb - ha, ccb), False, ha)
            else:
                neg = ((hb - ha, ccb), False, ha)
                pos = ((ha - hb, cca), True, hb)
            key, shift_a, base = pos if (pos[0] in keys and neg[0] not in keys) else neg
            keys[key] = keys.get(key, 0) + 1
            if shift_a:
                a_key, b_key = key, (0, ccb)
            else:
                a_key, b_key = (0, cca), key

        kab, kka, kkb, kk1 = float(cab[k]), float(ca[k]), float(cb[k]), float(c1[k])
        if abs(kab) <= 1e-7:
            path, scal, gamma = "linear", (kka, kkb, kk1), 0.0
        elif abs(kkb) <= 50.0 * abs(kab) and abs(kka * kkb) <= 50.0 * abs(kab):
            path = "fact"
            scal = (kab, kka, kkb / kab)
            gamma = kk1 - kka * kkb / kab
        else:
            path, scal, gamma = "exact", (kab, kka, kkb, kk1), 0.0
        raw.append((k, base, a_key, wa, b_key, wb, path, scal, gamma))

    # consolidate shifted columns into gap-bridged contiguous c-runs
    def build_runs(gaptol):
        runs, cmap, total = [], {}, 0
        for s in sorted({sc[0] for sc in keys}):
            cs = sorted(c for (s2, c) in keys if s2 == s)
            i = 0
            while i < len(cs):
                j = i
                while j + 1 < len(cs) and cs[j + 1] - cs[j] <= gaptol:
                    j += 1
                c0, cl = cs[i], cs[j]
                for c in range(c0, cl + 1):
                    cmap[(s, c)] = total + (c - c0)
                runs.append((s, c0, cl, total))
                total += cl - c0 + 1
                i = j + 1
        return runs, cmap, total

    for gaptol in (8, 4, 1, 0):
        runlist, cmap, ncols = build_runs(gaptol)
        if ncols <= 75:
            break

    plans = []
    for (k, base, a_key, wa, b_key, wb, path, scal, gamma) in raw:
        a_src = (False, a_key[1], wa) if a_key[0] == 0 else (True, cmap[a_key], wa)
        b_src = (False, b_key[1], wb) if b_key[0] == 0 else (True, cmap[b_key], wb)
        plans.append((k, base, a_src, b_src, path, scal, gamma))

    order = sorted(
        range(K), key=lambda k: (plans[k][1], plans[k][2][0] or plans[k][3][0], k)
    )  # by base, no-shift kernels first within each base run
    gcol = np.zeros((H, K), np.float32)
    for pos, k in enumerate(order):
        gcol[:, pos] = plans[k][6]
    return plans, (runlist, ncols), order, gcol


def _build(pairs_a, pairs_b, weights):
    import concourse.bacc as bacc
    import concourse.mybir as mybir
    from concourse.tile import TileContext

    f32 = mybir.dt.float32
    Copy = mybir.ActivationFunctionType.Copy
    add, mult = mybir.AluOpType.add, mybir.AluOpType.mult

    plans, (runlist, ncols), order, gcol_np = _plan(pairs_a, pairs_b, weights)
    ncols = max(1, ncols)
    ngrp = (K + GRP - 1) // GRP

    if ncols > 80:
        raise RuntimeError(f"shifted-column budget exceeded: {ncols}")

    nc = bacc.Bacc()
    # h-major DRAM layout: each SBUF partition row of the main load is one
    # contiguous 64KB DRAM span (the former [C,H,...] layout produced 1KB
    # descriptors and measured ~74 GB/s; h-major reaches near line rate).
    x = nc.dram_tensor("x", [H, C, BPC, W], f32, kind="ExternalInput")
    gcd = nc.dram_tensor("gcol", [H, K], f32, kind="ExternalInput")
    # oh-major output: a store from T[base:base+OH] writes, per partition,
    # one contiguous (k-run x BPC x OW) span (~8KB for a full group) instead
    # of 504B per (k,b).  Host transposes back.
    out = nc.dram_tensor("out", [OH, K, BPC, OW], f32, kind="ExternalOutput")

    with TileContext(nc) as tc:
        with (
            tc.tile_pool(name="xp", bufs=1) as xp,
            tc.tile_pool(name="bp", bufs=6) as bp,
            tc.tile_pool(name="tp", bufs=3) as tp,
            tc.tile_pool(name="op", bufs=2) as op,
        ):
            # x arrives host-transposed as [H, C, BPC, W]: per-partition
            # contiguous loads.  Chunked over c so compute on early channels
            # can start while later chunks stream; two chunks ride the
            # otherwise-idle Activation HWDGE ring in parallel with SP.
            xr = x.rearrange("h c b w -> h c (b w)")
            X = xp.tile([H, C * BPC * W], f32)
            Xv = X.rearrange("p (c b w) -> p c b w", c=C, b=BPC)
            Xf = X.rearrange("p (c q) -> p c q", c=C)
            qtr = C // 4
            for ci in range(4):
                eng = nc.sync if ci % 2 == 0 else nc.scalar
                eng.dma_start(
                    out=Xf[:, ci * qtr : (ci + 1) * qtr],
                    in_=xr[:, ci * qtr : (ci + 1) * qtr],
                )

            S = xp.tile([H, ncols * BPC * W], f32)
            Sv = S.rearrange("p (j b w) -> p j b w", j=ncols, b=BPC)
            Sf = S.rearrange("p (j q) -> p j q", j=ncols)
            # Shifted-run + filler + gcol loads go on the SWDGE (gpsimd)
            # queue: their 1-4KB / 256KB-strided descriptors all hash onto a
            # single SDMA engine under HWDGE (measured 23 GB/s); SWDGE
            # round-robins descriptors across all 16 engines.  GpSimd is idle
            # during the load window, and no DVE 2-port ops exist here to
            # starve SWDGE descriptor generation.
            # finite filler for shifted-run head/tail rows (junk lanes only)
            for d0 in range(0, ncols, C):
                n = min(C, ncols - d0)
                nc.gpsimd.dma_start(out=Sf[0:2, d0 : d0 + n], in_=xr[0:2, 0:n])
                nc.gpsimd.dma_start(out=Sf[H - 2 : H, d0 : d0 + n], in_=xr[0:2, 0:n])
            for ri, (s, c0, cl, d0) in enumerate(runlist):
                # S[p, d0+i] = x[c0+i, p+s], loaded from DRAM.
                eng = nc.gpsimd
                n = cl - c0 + 1
                if s < 0:
                    eng.dma_start(
                        out=Sf[-s:H, d0 : d0 + n], in_=xr[0 : H + s, c0 : c0 + n]
                    )
                else:
                    eng.dma_start(
                        out=Sf[0 : H - s, d0 : d0 + n], in_=xr[s:H, c0 : c0 + n]
                    )

            Gc = xp.tile([H, K], f32)
            nc.gpsimd.dma_start(out=Gc, in_=gcd[:, :])

            out_kb = out.rearrange("oh k b ow -> oh (k b) ow")
            fd = BPC * OW

            def emit_gamma_and_store(g, ks, geng, T, O):
                # deferred one group so cross-engine waits are pre-satisfied
                for j, k in enumerate(ks):
                    _, base, _, _, path, scal, gamma = plans[k]
                    cnt = base + OH
                    slot = T[0:cnt, j * fd : (j + 1) * fd]
                    if gamma != 0.0 or geng == "gp":
                        pos = g * GRP + j
                        if geng == "act":
                            nc.scalar.activation(
                                slot, slot, Copy, bias=gamma, scale=1.0
                            )
                        elif geng == "dve":
                            # odd innermost dim forces 1x mode: no shared-port
                            # contention with GpSimd
                            so = slot.rearrange("p (a q) -> p a q", a=4)
                            nc.vector.tensor_scalar(so, so, gamma, None, add)
                        else:
                            gb = Gc[0:cnt, pos : pos + 1].broadcast_to([cnt, fd])
                            osl = O[0:cnt, j * fd : (j + 1) * fd]
                            nc.gpsimd.tensor_tensor(osl, slot, gb, add)
                # batched stores per same-base run: SWDGE on the GpSimd queue
                # (issue ~0.7us, transfer async; HWDGE would block its engine).
                src_t = O if geng == "gp" else T
                i = 0
                while i < len(ks):
                    base = plans[ks[i]][1]
                    i2 = i
                    while i2 < len(ks) and plans[ks[i2]][1] == base:
                        i2 += 1
                    src = src_t[base : base + OH, i * fd : i2 * fd].rearrange(
                        "p (kb w) -> p kb w", w=OW
                    )
                    dst = out_kb[:, (g * GRP + i) * BPC : (g * GRP + i2) * BPC]
                    nc.gpsimd.dma_start(out=dst, in_=src)
                    i = i2

            pending = None
            for g in range(ngrp):
                ks = order[g * GRP : (g + 1) * GRP]
                geng = GSPLIT[g % len(GSPLIT)]
                T = tp.tile([H, GRP * fd], f32, tag="t", name=f"t_{g}")
                O = None
                if geng == "gp":
                    O = op.tile([H, GRP * fd], f32, tag="o", name=f"o_{g}")

                for j, k in enumerate(ks):
                    _, base, a_src, b_src, path, scal, gamma = plans[k]
                    cnt = base + OH

                    def view(src):
                        shifted, idx, woff = src
                        t = Sv if shifted else Xv
                        return t[0:cnt, idx, :, woff : woff + OW]

                    Av, Bv = view(a_src), view(b_src)
                    slot = T[0:cnt, j * fd : (j + 1) * fd]
                    slotv = slot.rearrange("p (b w) -> p b w", b=BPC)
                    b2 = bp.tile([H, fd], f32, tag="b2", name=f"b2_{k}")
                    b2v = b2.rearrange("p (b w) -> p b w", b=BPC)[0:cnt]

                    if path == "fact":
                        kab, kka, alpha = scal
                        nc.scalar.activation(b2v, Bv, Copy, bias=kka, scale=kab)
                        nc.vector.scalar_tensor_tensor(slotv, Av, alpha, b2v, add, mult)
                    else:  # linear/exact: slot = Ca*A + (Cb*B + C1)
                        if path == "linear":
                            kka, kkb, kk1 = scal
                        else:
                            kab, kka, kkb, kk1 = scal
                        nc.scalar.activation(b2v, Bv, Copy, bias=kk1, scale=kkb)
                        nc.vector.scalar_tensor_tensor(slotv, Av, kka, b2v, mult, add)
                        if path == "exact":  # += (Cab*B)*A
                            p2 = bp.tile([H, fd], f32, tag="b2", name=f"p2_{k}")
                            p2v = p2.rearrange("p (b w) -> p b w", b=BPC)[0:cnt]
                            nc.vector.scalar_tensor_tensor(p2v, Bv, kab, Av, mult, mult)
                            nc.vector.tensor_tensor(slot, slot, p2[0:cnt], add)

                if pending is not None:
                    emit_gamma_and_store(*pending)
                pending = (g, ks, geng, T, O)
            if pending is not None:
                emit_gamma_and_store(*pending)
    nc.compile()
    return nc


def _consts(pairs_a, pairs_b, weights):
    plans, runs, order, gcol = _plan(pairs_a, pairs_b, weights)
    return {"gcol": gcol}, order


def kernel(x, pairs_a, pairs_b, weights):
    from concourse.bass_utils import run_bass_kernel_spmd

    x = np.ascontiguousarray(np.asarray(x), dtype=np.float32)
    pa = np.asarray(pairs_a).astype(np.int64)
    pb = np.asarray(pairs_b).astype(np.int64)
    w = np.asarray(weights).astype(np.float32)

    nc = _build(pa, pb, w)
    extra, order = _consts(pa, pb, w)
    in_maps = [
        {
            "x": np.ascontiguousarray(
                x[i * BPC : (i + 1) * BPC].transpose(2, 1, 0, 3)
            ),
            **extra,
        }
        for i in range(NCORES)
    ]
    res = run_bass_kernel_spmd(nc, in_maps, core_ids=list(range(NCORES)))
    # device layout [OH, K(sorted), BPC, OW] per core -> [B, K, OH, OW]
    full = np.concatenate([r["out"] for r in res.results], axis=2)  # [OH,K,B,OW]
    pos = np.empty(K, np.int64)
    pos[np.asarray(order)] = np.arange(K)
    return np.ascontiguousarray(full[:, pos].transpose(2, 1, 0, 3))

